# revision 7
# baseline (speedup 1.0000x reference)
"""DiagGCN message-passing kernel for 8 Trainium2 NeuronCores.

Strategy (receiver-sharded, no collectives):
  - Core c owns output rows [c*12500, (c+1)*12500). Edges are bucketed to
    cores by recv_idx, so each core computes its output slice completely.
  - Within a core, edges are ordered by (sender-chunk, 128-node window of
    the receiver). Sender chunks (4 x 25000 rows) keep dma_gather indices
    within int16 range. Subgroup sizes are padded to a cross-core-uniform
    schedule so one SPMD program serves all 8 cores; pad slots carry
    weight 0 and contribute nothing.
  - Per span of up to SPAN edge slots: dma_gather sender rows (512B each)
    from HBM into SBUF (edge e -> partition e%128).
  - Type rows are NOT gathered (the relation table is only 401 rows, and
    per-edge gather descriptors dominate profiled runtime). Instead the
    table lives in SBUF (bf16, zero-padded to 512 rows) and per-edge type
    rows are expanded with the tensor engine: a rank-1 matmul broadcasts
    each edge's type id across partitions, DVE is_equal builds a
    [type, edge] one-hot per 128-type chunk, and 4 accumulating bf16
    matmuls produce T[t_e, :] per edge tile in PSUM.
  - DVE: msg = sender * typerow (+bias); ACT: relu; DVE: build weighted
    one-hot lhsT[e, m] = w[e] * (recv_inwin[e] == m) in ONE fused
    tensor_scalar(is_equal, mult) op per 128-edge tile.
  - PE: psum[window] (+)= onehot^T @ msg  (segment-sum as matmul).
  - DVE drains each finished window from PSUM into an SBUF accumulator;
    one strided DMA writes the [12500, 128] slice at the end.
  - PSUM budget: banks 0-3 aggregation rotation, 4-5 type-id broadcast,
    6-7 expanded type rows.
"""
import sys
sys.path.insert(0, "/opt/trn_rl_repo")
import numpy as np
import ml_dtypes
from dataclasses import dataclass


@dataclass(frozen=True)
class Config:
    n_nodes: int = 100000
    n_edges: int = 600000
    d: int = 128
    n_types: int = 401
    n_cores: int = 8
    chunks: int = 4          # sender-table chunks (int16 idx limit)
    span: int = 1024         # edge slots per gather call / compute span
    win: int = 512           # receiver window (matmul free dim)

    @property
    def npc(self):           # nodes per core
        return self.n_nodes // self.n_cores

    @property
    def nwin(self):          # windows per core
        return (self.npc + self.win - 1) // self.win

    @property
    def crows(self):         # sender rows per chunk
        return (self.n_nodes + self.chunks - 1) // self.chunks


CFG = Config()
TCH = 4                      # type chunks of 128 (401 -> 512 padded)

_PROGRAM_CACHE = {}


def _wrap16(arr):
    """[NC, L] int -> [NC, 128, L/16] int16: idx j at [:, j%16, j//16], x8."""
    nc_, L = arr.shape
    a = arr.astype(np.int16).reshape(nc_, L // 16, 16).transpose(0, 2, 1)
    return np.ascontiguousarray(np.tile(a, (1, 8, 1)))


def _wrap128(arr):
    """[NC, L] f32 -> [NC, 128, L/128]: slot j at [:, j%128, j//128]."""
    nc_, L = arr.shape
    a = arr.astype(np.float32).reshape(nc_, L // 128, 128).transpose(0, 2, 1)
    return np.ascontiguousarray(a)


def _schedule(S, cfg):
    """Static schedule from padded subgroup sizes S [chunks, nwin]."""
    chunks, nwin = S.shape
    offs = np.concatenate([[0], np.cumsum(S.ravel())])[:-1].reshape(chunks, nwin)
    L = int(S.sum())
    # window modes: first nonempty chunk copies, later chunks add
    first_chunk = np.full(nwin, -1, np.int64)
    for c in range(chunks):
        m = (S[c] > 0) & (first_chunk < 0)
        first_chunk[m] = c
    memset_windows = [w for w in range(nwin) if first_chunk[w] < 0]

    spans = []   # (slot_off, n_slots, chunk, span_tiles)
    gw = -1
    windows = []  # per nonempty (c,w): dict(c,w,gw,mode)
    for c in range(chunks):
        Lc = int(S[c].sum())
        if Lc == 0:
            continue
        c_off = int(offs[c, 0])
        # tiles of this chunk in order, annotated with window + first/last
        tiles = []
        for w in range(nwin):
            nt = int(S[c, w]) // 128
            if nt == 0:
                continue
            gw += 1
            windows.append(dict(c=c, w=w, gw=gw,
                                mode="copy" if first_chunk[w] == c else "add"))
            for k in range(nt):
                tiles.append(dict(w=w, gw=gw, first=(k == 0), last=(k == nt - 1)))
        # split into spans
        pos = 0
        while pos < Lc:
            n = min(cfg.span, Lc - pos)
            t0 = pos // 128
            spans.append(dict(off=c_off + pos, n=n, chunk=c,
                              tiles=tiles[t0:t0 + n // 128]))
            pos += n
    # windows ending per span index
    for s, sp in enumerate(spans):
        sp["ending"] = [t["gw"] for t in sp["tiles"] if t["last"]]
    return dict(spans=spans, windows=windows, memset_windows=memset_windows,
                L=L, offs=offs, n_windows=gw + 1)


def _build_program(S_bytes, L, has_bias, cfg):
    import concourse.bacc as bacc
    import concourse.bass as bass
    import concourse.mybir as mybir
    from concourse.library_config import mlp

    S = np.frombuffer(S_bytes, np.int64).reshape(cfg.chunks, cfg.nwin)
    sch = _schedule(S, cfg)
    spans, windows = sch["spans"], sch["windows"]
    nspan = len(spans)
    n_windows = sch["n_windows"]
    NWIN, NPC, D, WIN = cfg.nwin, cfg.npc, cfg.d, cfg.win
    SPAN_T = cfg.span // 128
    f32 = mybir.dt.float32
    f32r = mybir.dt.float32r
    bf16 = mybir.dt.bfloat16

    nc = bacc.Bacc("TRN2", debug=True, num_swdge_queues=4)
    vtab = nc.dram_tensor("vtab", [cfg.n_nodes, D], f32, kind="ExternalInput")
    sidx_d = nc.dram_tensor("sidx", [128, L // 16], mybir.dt.int16, kind="ExternalInput")
    recvf_d = nc.dram_tensor("recvf", [128, L // 128], f32, kind="ExternalInput")
    wf_d = nc.dram_tensor("wf", [128, L // 128], f32, kind="ExternalInput")
    iota_d = nc.dram_tensor("iota", [128, WIN], f32, kind="ExternalInput")
    vtypb_d = nc.dram_tensor("vtypb", [128, TCH * D], bf16, kind="ExternalInput")
    tfw_d = nc.dram_tensor("tfw", [128, L // 128], f32, kind="ExternalInput")
    ident_d = nc.dram_tensor("ident", [128, 128], f32, kind="ExternalInput")
    iott_d = nc.dram_tensor("iott", [128, TCH + 1], f32, kind="ExternalInput")
    ones_d = nc.dram_tensor("ones", [1, 128], f32r, kind="ExternalInput")
    if has_bias:
        brep_d = nc.dram_tensor("brep", [128, D], f32, kind="ExternalInput")
    out_d = nc.dram_tensor("out", [128, NWIN * WIN], f32, kind="ExternalOutput")

    from contextlib import ExitStack
    with ExitStack() as ctx:
        sidx_t = ctx.enter_context(nc.sbuf_tensor("sidx_t", [128, L // 16], mybir.dt.int16))
        recvf_t = ctx.enter_context(nc.sbuf_tensor("recvf_t", [128, L // 128], f32))
        wf_t = ctx.enter_context(nc.sbuf_tensor("wf_t", [128, L // 128], f32))
        iota_t = ctx.enter_context(nc.sbuf_tensor("iota_t", [128, WIN], f32))
        vtypb_t = ctx.enter_context(nc.sbuf_tensor("vtypb_t", [128, TCH, D], bf16))
        tfw_t = ctx.enter_context(nc.sbuf_tensor("tfw_t", [128, L // 128], f32))
        ident_t = ctx.enter_context(nc.sbuf_tensor("ident_t", [128, 128], f32))
        trow_row = ctx.enter_context(nc.sbuf_tensor("trow_row", [1, cfg.span], f32))
        iott_t = ctx.enter_context(nc.sbuf_tensor("iott_t", [128, TCH + 1], f32))
        ones_t = ctx.enter_context(nc.sbuf_tensor("ones_t", [1, 128], f32r))
        brep_t = ctx.enter_context(nc.sbuf_tensor("brep_t", [128, D], f32))
        NBUF = 4
        sbufs = [ctx.enter_context(nc.sbuf_tensor(f"sbuf{i}", [128, SPAN_T, D], f32))
                 for i in range(NBUF)]
        OHBUF = 3
        ohbufs = [ctx.enter_context(nc.sbuf_tensor(f"ohbuf{i}", [128, SPAN_T, WIN], f32))
                  for i in range(OHBUF)]
        rbufs = [ctx.enter_context(nc.sbuf_tensor(f"rbuf{i}", [128, SPAN_T, D], f32))
                 for i in range(OHBUF)]
        OTBUF = 2
        otbufs = [ctx.enter_context(
            nc.sbuf_tensor(f"otbuf{i}", [128, SPAN_T, TCH, D], bf16))
            for i in range(OTBUF)]
        accum = ctx.enter_context(nc.sbuf_tensor("accum", [128, NWIN * WIN], f32))
        psum = ctx.enter_context(nc.psum_tensor("psum", [128, 8, 512], f32))
        ld = ctx.enter_context(nc.semaphore("ld"))
        sq0 = ctx.enter_context(nc.semaphore("sq0"))
        sq1 = ctx.enter_context(nc.semaphore("sq1"))
        vm = ctx.enter_context(nc.semaphore("vm"))
        ar = ctx.enter_context(nc.semaphore("ar"))
        ohs = ctx.enter_context(nc.semaphore("ohs"))
        trs = ctx.enter_context(nc.semaphore("trs"))
        trd = ctx.enter_context(nc.semaphore("trd"))
        bcs = ctx.enter_context(nc.semaphore("bcs"))
        oht = ctx.enter_context(nc.semaphore("oht"))
        tss = ctx.enter_context(nc.semaphore("tss"))
        mm = ctx.enter_context(nc.semaphore("mm"))
        rs = ctx.enter_context(nc.semaphore("rs"))
        pes = ctx.enter_context(nc.semaphore("pes"))
        od = ctx.enter_context(nc.semaphore("od"))
        fin = ctx.enter_context(nc.semaphore("fin"))
        block = ctx.enter_context(nc.Block())
        n_loads = 10 if has_bias else 9

        # 4-bank aggregation rotation safety: when PE starts window gw (at
        # span s), windows <= gw-4 must already be drained; drains lag two
        # spans behind DVE. Verify statically for this schedule.
        ends_through = np.zeros(nspan + 1, np.int64)  # endings in spans < s
        for s_i, sp in enumerate(spans):
            ends_through[s_i + 1] = ends_through[s_i] + len(sp["ending"])
        for s_i, sp in enumerate(spans):
            for tile in sp["tiles"]:
                if tile["first"] and tile["gw"] >= 4:
                    avail = ends_through[max(s_i - 1, 0)]  # drained: spans <= s-2
                    assert avail >= tile["gw"] - 3, (
                        f"agg psum rotation hazard: span {s_i} window {tile['gw']}"
                    )

        @block.gpsimd
        def _(g):
            g.load_library(mlp)
            g.dma_start(sidx_t[:], sidx_d[:]).then_inc(ld, 16)
            g.dma_start(recvf_t[:], recvf_d[:]).then_inc(ld, 16)
            g.dma_start(wf_t[:], wf_d[:]).then_inc(ld, 16)
            g.dma_start(iota_t[:], iota_d[:]).then_inc(ld, 16)
            g.dma_start(vtypb_t[:], vtypb_d[:].rearrange("p (c d) -> p c d", c=TCH)).then_inc(ld, 16)
            g.dma_start(tfw_t[:], tfw_d[:]).then_inc(ld, 16)
            g.dma_start(ident_t[:], ident_d[:]).then_inc(ld, 16)
            g.dma_start(iott_t[:], iott_d[:]).then_inc(ld, 16)
            g.dma_start(ones_t[:], ones_d[:]).then_inc(ld, 16)
            if has_bias:
                g.dma_start(brep_t[:], brep_d[:]).then_inc(ld, 16)
            # gather desc-gen only reads sidx (first load); the aux loads
            # overlap the first gathers.
            g.wait_ge(ld, 16)
            for s, sp in enumerate(spans):
                if s >= NBUF:
                    g.wait_ge(pes, s - NBUF + 1)
                k, n, off, c = s % NBUF, sp["n"], sp["off"], sp["chunk"]
                nt = n // 128
                cr0 = c * cfg.crows
                crn = min(cfg.crows, cfg.n_nodes - cr0)
                p = s % 2
                ssem = sq0 if p == 0 else sq1
                if s >= 2:
                    g.wait_ge(ssem, 16 * (s // 2))
                g.dma_gather(
                    sbufs[k][:, :nt, :], vtab[cr0:cr0 + crn, :],
                    sidx_t[:, off // 16:(off + n) // 16], n, n, D,
                    queue_num=p,
                ).then_inc(ssem, 16)
            # final output store (host transposes [128, nodes] -> [nodes, 128])
            g.wait_ge(rs, n_windows)
            g.wait_ge(fin, 1)
            g.dma_start(out_d[:], accum[:]).then_inc(od, 16)
            g.wait_ge(od, 16)

        @block.vector
        def _(v):
            v.wait_ge(ld, 16 * n_loads)
            for w in sch["memset_windows"]:
                v.memset(accum[:, w * WIN:(w + 1) * WIN], 0.0)

            def drain_window(wi):
                win = windows[wi]
                v.wait_ge(mm, wi + 1)
                dst = accum[:, win["w"] * WIN:(win["w"] + 1) * WIN]
                src = psum[:, wi % 4, :]
                if win["mode"] == "copy":
                    v.tensor_copy(dst, src)
                else:
                    v.tensor_add(dst, dst, src)
                v.sem_inc(rs, 1)
            for s, sp in enumerate(spans):
                k, n, off = s % NBUF, sp["n"], sp["off"]
                ko = s % OHBUF
                kt = s % OTBUF
                nt = n // 128
                if s >= OHBUF:
                    v.wait_ge(pes, s - OHBUF + 1)
                # drain windows finished TWO spans back: PE is far ahead so
                # the mm wait is already satisfied
                if s >= 2:
                    for wi in spans[s - 2]["ending"]:
                        drain_window(wi)
                # receiver one-hots depend only on preloaded recvf/wf: build
                # them while this span's gathers are still in flight. fp32r
                # out: the fp32r matmul requires rounded producers.
                for i in range(nt):
                    col = off // 128 + i
                    v.tensor_scalar(
                        ohbufs[ko][:, i, :].bitcast(f32r),
                        iota_t[:, :].rearrange("p (o d) -> p o d", o=1),
                        recvf_t[:, col:col + 1],
                        wf_t[:, col:col + 1],
                        mybir.AluOpType.is_equal,
                        mybir.AluOpType.mult,
                    )
                v.drain().then_inc(ohs, 1)
                # drain the PE's per-tile type-id transposes (partition-0
                # rows of banks 4-5) into an SBUF row for the broadcast mm
                v.wait_ge(trs, s + 1)
                v.tensor_copy(trow_row[0:1, :].bitcast(f32r), psum[0:1, 4:6, :])
                v.drain().then_inc(trd, 1)
                # type one-hots [t, e] from the PE's type-id broadcast.
                # otbuf reused mod 2: PE consumed it at span s-2 (tss).
                v.wait_ge(bcs, s + 1)
                if s >= OTBUF:
                    v.wait_ge(tss, s - OTBUF + 1)
                for i in range(nt):
                    bc = psum[:, 4 + i // 4, (i % 4) * 128:(i % 4) * 128 + 128]
                    for c in range(TCH):
                        v.tensor_scalar(
                            otbufs[kt][:, i, c, :],
                            bc.rearrange("p (o d) -> p o d", o=1),
                            iott_t[:, c:c + 1],
                            iott_t[:, TCH:TCH + 1],
                            mybir.AluOpType.is_equal,
                            mybir.AluOpType.mult,
                        )
                v.drain().then_inc(oht, 1)
                v.wait_ge(sq0, 16 * (s // 2 + 1))
                if s >= 1:
                    v.wait_ge(sq1, 16 * ((s - 1) // 2 + 1))
                # multiply gathered sender rows by expanded type rows (PSUM
                # banks 6-7, 4 tiles each)
                v.wait_ge(tss, s + 1)
                v.tensor_mul(sbufs[k][:, :min(nt, 4), :], sbufs[k][:, :min(nt, 4), :],
                             psum[:, 6, :].rearrange("p (o d) -> p o d", o=SPAN_T // 2)[:, :min(nt, 4), :])
                if nt > 4:
                    v.tensor_mul(sbufs[k][:, 4:nt, :], sbufs[k][:, 4:nt, :],
                                 psum[:, 7, :].rearrange("p (o d) -> p o d", o=SPAN_T // 2)[:, :nt - 4, :])
                if has_bias:
                    for i in range(nt):
                        v.tensor_add(sbufs[k][:, i, :], sbufs[k][:, i, :],
                                     brep_t[:, :].rearrange("p (o d) -> p o d", o=1))
                v.drain().then_inc(vm, 1)
            for sp in spans[-2:]:
                for wi in sp["ending"]:
                    drain_window(wi)
            v.drain().then_inc(fin, 1)

        @block.scalar
        def _(a):
            for s, sp in enumerate(spans):
                k, nt = s % NBUF, sp["n"] // 128
                ko = s % OHBUF
                if s >= OHBUF:
                    a.wait_ge(pes, s - OHBUF + 1)
                a.wait_ge(vm, s + 1)
                a.activation(rbufs[ko][:, :nt, :].bitcast(f32r),
                             sbufs[k][:, :nt, :],
                             mybir.ActivationFunctionType.Relu)
                a.drain().then_inc(ar, 1)

        @block.tensor
        def _(t):
            t.wait_ge(ld, 16 * n_loads)
            for s, sp in enumerate(spans):
                k = s % NBUF
                ko = s % OHBUF
                kt = s % OTBUF
                nt = sp["n"] // 128
                # per-tile type-id column -> partition-0 row (banks 4-5);
                # DVE finished reading those banks for span s-1 (oht).
                if s >= 1:
                    t.wait_ge(oht, s)
                for i in range(nt):
                    col = sp["off"] // 128 + i
                    t.transpose(
                        psum[0:1, 4 + i // 4, (i % 4) * 128:(i % 4) * 128 + 128],
                        tfw_t[:, col:col + 1],
                        ident_t[:, :],
                    )
                t.drain().then_inc(trs, 1)
                # broadcast the drained row across partitions into banks 4-5
                t.wait_ge(trd, s + 1)
                nhalf = (nt + 3) // 4
                for h in range(nhalf):
                    t.matmul(
                        psum[:, 4 + h, :],
                        ones_t[:, :],
                        trow_row[0:1, h * 512:h * 512 + 512].bitcast(f32r),
                        start=True, stop=True,
                    )
                t.drain().then_inc(bcs, 1)
                # expanded type rows into banks 6-7 (4 chunk matmuls/tile).
                # DVE read banks 6-7 for span s-1 during its mult (vm).
                t.wait_ge(oht, s + 1)
                if s >= 1:
                    t.wait_ge(vm, s)
                for i in range(nt):
                    dst = psum[:, 6 + i // 4, (i % 4) * 128:(i % 4) * 128 + 128]
                    for c in range(TCH):
                        t.matmul(
                            dst,
                            otbufs[kt][:, i, c, :],
                            vtypb_t[:, c, :],
                            start=(c == 0), stop=(c == TCH - 1),
                        )
                t.drain().then_inc(tss, 1)
                t.wait_ge(ar, s + 1)
                t.wait_ge(ohs, s + 1)
                for i, tile in enumerate(sp["tiles"]):
                    gw = tile["gw"]
                    if tile["first"] and gw >= 4:
                        t.wait_ge(rs, gw - 3)
                    inst = t.matmul(
                        psum[:, gw % 4, :],
                        rbufs[ko][:, i, :].bitcast(f32r),
                        ohbufs[ko][:, i, :].bitcast(f32r),
                        start=tile["first"], stop=tile["last"],
                    )
                    if tile["last"]:
                        inst.then_inc(mm, 1)
                t.drain().then_inc(pes, 1)

    nc.compile()
    return nc


def _get_program(S, L, has_bias, cfg):
    key = (S.tobytes(), L, has_bias, cfg)
    if key not in _PROGRAM_CACHE:
        _PROGRAM_CACHE[key] = _build_program(S.tobytes(), L, has_bias, cfg)
    return _PROGRAM_CACHE[key]


def _prepare(V, VT, B, w, snd, typ, rcv, cfg):
    NC, NPC, NWIN, CH = cfg.n_cores, cfg.npc, cfg.nwin, cfg.chunks
    E = cfg.n_edges
    snd = np.asarray(snd, np.int64)
    typ = np.asarray(typ, np.int64)
    rcv = np.asarray(rcv, np.int64)
    w = np.asarray(w, np.float32)

    core = rcv // NPC
    rloc = rcv - core * NPC
    win = rloc // cfg.win
    rin = (rloc - win * cfg.win).astype(np.float32)
    chunk = snd // cfg.crows
    sloc = (snd - chunk * cfg.crows).astype(np.int16)

    key = (core * CH + chunk) * NWIN + win
    # secondary sort by sender id: gather descriptors hit ascending HBM
    # addresses within each subgroup (DRAM row-buffer locality)
    order = np.lexsort((sloc, key))
    cnt = np.bincount(key, minlength=NC * CH * NWIN).reshape(NC, CH, NWIN)
    S = ((cnt.max(axis=0) + 127) // 128 * 128).astype(np.int64)  # [CH, NWIN]
    offs = np.concatenate([[0], np.cumsum(S.ravel())])[:-1].reshape(CH, NWIN)
    L = int(S.sum())

    # slot for each edge (in sorted order)
    cnt_flat = cnt.ravel()
    grp_start = np.concatenate([[0], np.cumsum(cnt_flat)])[:-1]
    ranks = np.arange(E) - np.repeat(grp_start, cnt_flat)
    # per-edge (sorted) subgroup offset: offs[chunk, win] (same for all cores)
    ids = key[order]
    c_of = (ids // NWIN) % CH
    w_of = ids % NWIN
    slot = offs[c_of, w_of] + ranks
    core_s = core[order]

    sl_s = np.zeros((NC, L), np.int16)
    sl_t = np.zeros((NC, L), np.float32)
    sl_r = np.zeros((NC, L), np.float32)
    sl_w = np.zeros((NC, L), np.float32)
    sl_s[core_s, slot] = sloc[order]
    sl_t[core_s, slot] = typ[order].astype(np.float32)
    sl_r[core_s, slot] = rin[order]
    sl_w[core_s, slot] = w[order]

    iota = np.ascontiguousarray(
        np.tile(np.arange(cfg.win, dtype=np.float32), (128, 1)))
    # type one-hot per-partition scalars: col c = 128c + p, last col = 1.0
    iott = np.empty((128, TCH + 1), np.float32)
    for c in range(TCH):
        iott[:, c] = 128 * c + np.arange(128)
    iott[:, TCH] = 1.0
    ones = np.ones((1, 128), np.float32)
    ident = np.eye(128, dtype=np.float32)
    # type table, bf16, padded to 512 rows: vtypb[p, c, d] = VT[128c+p, d]
    VTp = np.zeros((TCH * 128, cfg.d), np.float32)
    VTp[:cfg.n_types] = np.asarray(VT, np.float32)
    vtypb = np.ascontiguousarray(
        VTp.reshape(TCH, 128, cfg.d).transpose(1, 0, 2).reshape(128, TCH * cfg.d)
    ).astype(ml_dtypes.bfloat16)

    has_bias = bool(np.any(np.asarray(B) != 0))
    in_maps = []
    sidx_w = _wrap16(sl_s)
    tfw = _wrap128(sl_t)
    recvf = _wrap128(sl_r)
    wf = _wrap128(sl_w)
    Vc = np.ascontiguousarray(np.asarray(V, np.float32))
    for i in range(NC):
        m = {"vtab": Vc, "sidx": sidx_w[i], "recvf": recvf[i], "wf": wf[i],
             "iota": iota, "vtypb": vtypb, "tfw": tfw[i],
             "iott": iott, "ones": ones, "ident": ident}
        if has_bias:
            m["brep"] = np.ascontiguousarray(
                np.tile(np.asarray(B, np.float32), (128, 1)))
        in_maps.append(m)
    return S, L, has_bias, in_maps


def _run(V, VT, B, w, snd, typ, rcv, cfg=None, trace=False):
    from concourse.bass_utils import run_bass_kernel_spmd
    cfg = cfg or CFG
    S, L, has_bias, in_maps = _prepare(V, VT, B, w, snd, typ, rcv, cfg)
    nc = _get_program(S, L, has_bias, cfg)
    res = run_bass_kernel_spmd(nc, in_maps, list(range(cfg.n_cores)),
                               trace=trace)
    out = np.concatenate(
        [res.results[i]["out"][:, :cfg.npc].T for i in range(cfg.n_cores)], 0)
    return np.ascontiguousarray(out[:cfg.n_nodes]), res


def kernel(V_proj_sender, V_types, B_message, inc_weights,
           sender_idx, type_idx, recv_idx):
    out, _ = _run(V_proj_sender, V_types, B_message, inc_weights,
                  sender_idx, type_idx, recv_idx)
    return out
